# revision 21
# baseline (speedup 1.0000x reference)
"""Slot-attention kernel for Trainium2, SPMD over 8 NeuronCores.

Reference computation (per batch element b):
  query[b,n,:] = q[n,b,:] @ qw[n]          (n = 32 query slots)
  keyp [b,m,:] = k[m,b,:] @ kw[m]          (m = 32 key slots)
  value[b,m,:] = k[m,b,:] @ vw[m]
  logits[b,n,m] = query[b,n,:]·keyp[b,m,:] / 16
  attn = softmax_m(logits)
  out[n,b,:] = sum_m attn[b,n,m] * value[b,m,:]

Sharding: data-parallel over batch (4096 -> 512 per core), weights replicated.

Host-side prep packs everything into DMA-contiguous layouts (large per-
partition runs) so every DMA moves >=384KB of linear DRAM:
  combo[h, slot, p, kind, c, x]: per (half, slot) one 384KB DMA carrying
    {qT-slice, kT-slice, qw/16} with d%128 on partitions.
  kvw[g4, p, s, c, w, a]: key/value weights, resident in SBUF (64KB/part).
  out: written back as [h, sh, gc, p, gg, o] (contiguous 512KB stores);
    host untangles to [nq, bs, o].

Per-core schedule (two 256-batch halves; phases B/C per 128-batch sub-half):
  A) per-slot projections, full-clock N=256 matmuls; Q/K psum -> SBUF slot
     copies (ACT/DVE); V psum pairs -> VO[b][o,m] via strided-out copy, then
     one DVE 32x32 block-transpose per sub-half gives V32T[32r+m][o,g]
     (value with key-slot on partitions) -- no SWDGE shuffle.
  B) logits via 4x col-tiled 32x32 matmuls (batch = 32j+g), exp on ACT,
     rowsum+recip+normalize on DVE.
  C) attn^T via DVE 32x32 transpose; attn@value as 4-way diagonal-tiled
     matmuls with strided V32T rhs; psum quad copies -> OUTo; 512KB
     SWDGE stores on the otherwise-idle GpSimd queue.
"""

import numpy as np
import ml_dtypes

import concourse.bass as bass
from concourse import bacc
import concourse.mybir as mybir
import concourse.tile as tile
from concourse.bass_utils import run_bass_kernel_spmd

BF16 = mybir.dt.bfloat16
F32 = mybir.dt.float32

NQ = 32          # query slots
NK = 32          # key slots
D = 256          # input dim (contraction of projections)
A = 256          # attn dim (contraction of logits)
O = 256          # out dim
BS = 4096
N_CORES = 8
BS_CORE = BS // N_CORES   # 512
B_H = 256                 # batch per half
B_S = 128                 # batch per sub-half


def build_kernel():
    nc = bacc.Bacc()

    # combo[h, slot, p, kind(q,k), c, 256]
    comboD = nc.declare_dram_parameter("combo", [2, NQ, 128, 2, 2, 256], BF16,
                                       isOutput=False)
    # qw[slot, p, c, a] (pre-scaled by 1/16), streamed once per half
    qwD = nc.declare_dram_parameter("qw", [NQ, 128, 2, 256], BF16,
                                    isOutput=False)
    # kvw[g4, p, s4, c, w(k,v), a]
    kvwD = nc.declare_dram_parameter("kvw", [8, 128, 4, 2, 2, A], BF16,
                                     isOutput=False)
    # out[h, sh, gc, p(32j+n), gg, o]
    outD = nc.declare_dram_parameter("out", [2, 2, 4, 128, 8, O], BF16,
                                     isOutput=True)

    with tile.TileContext(nc) as tc:
        with (
            tc.tile_pool(name="const", bufs=1) as const_pool,
            tc.tile_pool(name="xin", bufs=3) as xin,
            tc.tile_pool(name="qwp", bufs=2) as qwp,
            tc.tile_pool(name="v32tmp", bufs=2) as v32tmp,
            tc.tile_pool(name="big", bufs=1) as big,
            tc.tile_pool(name="vop", bufs=2) as vop,
            tc.tile_pool(name="v32p", bufs=2) as v32p,
            tc.tile_pool(name="outp", bufs=2) as outp,
            tc.tile_pool(name="ep", bufs=1) as ep,
            tc.tile_pool(name="tep", bufs=2) as tep,
            tc.tile_pool(name="rsp", bufs=1) as rsp,
            tc.tile_pool(name="qk_ps", bufs=2, space="PSUM") as qk_ps,
            tc.tile_pool(name="vp_ps", bufs=2, space="PSUM") as vp_ps,
            tc.tile_pool(name="lg_ps", bufs=2, space="PSUM") as lg_ps,
        ):
            # resident K/V weights: [p, slot, c, (kw|vw), a]
            KVW = const_pool.tile([128, NK, 2, 2, A], BF16, tag="KVW")

            # ~4us of dummy back-to-back matmuls while the first input DMAs
            # are in flight, so the PE_HAM clock gate reaches K=8/8 before
            # the real work starts
            warm = tep.tile([128, 8, 32], BF16, tag="te8", name="warm")
            nc.vector.memset(warm.rearrange("p a b -> p (a b)"), 0.0)
            wsrc = warm.rearrange("p a b -> p (a b)")[:, 0:128]
            wps = lg_ps.tile([128, 16, 32], F32, tag="lg", name="warmps")
            for _ in range(44):
                nc.tensor.matmul(wps[:, 0:4, :], lhsT=wsrc, rhs=wsrc,
                                 start=True, stop=True)

            # K/V weight residency loads (1MB each) on the scalar HWDGE ring
            for gk in range(8):
                nc.scalar.dma_start(
                    out=KVW[:, 4 * gk:4 * gk + 4, :, :, :], in_=kvwD[gk])

            for h in range(2):
                # ---- Phase A: projections ----
                QTs = big.tile([128, NQ, 2, B_H], BF16, tag="QTs")
                KTs = big.tile([128, NK, 2, B_H], BF16, tag="KTs")
                # VN[sh][b%128, m, o] = value[b][m, o]  (natural layout)
                VN = [vop.tile([128, NK, O], BF16, tag="VN",
                               name=f"VN_{h}_{sh}") for sh in range(2)]

                vps = None
                for g in range(NQ):
                    cb = xin.tile([128, 2, 2, 256], BF16, tag="cb")
                    qwt = qwp.tile([128, 2, 256], BF16, tag="qwt")
                    # alternate HWDGE(sync) / SWDGE(gpsimd) queues for
                    # input-stream parallelism
                    if g % 2 == 0:
                        nc.sync.dma_start(out=cb, in_=comboD[h, g])
                        nc.gpsimd.dma_start(out=qwt, in_=qwD[g])
                    else:
                        nc.gpsimd.dma_start(out=cb, in_=comboD[h, g])
                        nc.sync.dma_start(out=qwt, in_=qwD[g])

                    # Q projection: psum [a%128, t, b], qw pre-scaled by 1/16
                    qps = qk_ps.tile([128, 2, B_H], F32, tag="qk")
                    for t in range(2):
                        for c in range(2):
                            nc.tensor.matmul(
                                qps[:, t, :],
                                lhsT=qwt[:, c, t * 128:(t + 1) * 128],
                                rhs=cb[:, 0, c, :],
                                start=(c == 0), stop=(c == 1))
                    nc.scalar.copy(out=QTs[:, g, :, :], in_=qps)
                    # K projection
                    kps = qk_ps.tile([128, 2, B_H], F32, tag="qk")
                    for t in range(2):
                        for c in range(2):
                            nc.tensor.matmul(
                                kps[:, t, :],
                                lhsT=KVW[:, g, c, 0, t * 128:(t + 1) * 128],
                                rhs=cb[:, 1, c, :],
                                start=(c == 0), stop=(c == 1))
                    if g % 2 == 0:
                        nc.vector.tensor_copy(out=KTs[:, g, :, :], in_=kps)
                    else:
                        nc.scalar.copy(out=KTs[:, g, :, :], in_=kps)
                    # V projection: psum [b%128, s-pair, sh, o]
                    sp = g % 2
                    if sp == 0:
                        vps = vp_ps.tile([128, 2, 2, O], F32, tag="vp")
                    for sh in range(2):
                        for c in range(2):
                            nc.tensor.matmul(
                                vps[:, sp, sh, :],
                                lhsT=cb[:, 1, c, sh * 128:(sh + 1) * 128],
                                rhs=KVW[:, g, c, 1, :],
                                start=(c == 0), stop=(c == 1))
                    if sp == 1:
                        g0 = g - 1
                        # contiguous pair copies into the natural V layout
                        nc.scalar.copy(out=VN[0][:, g0:g0 + 2, :],
                                       in_=vps[:, :, 0, :])
                        nc.vector.tensor_copy(out=VN[1][:, g0:g0 + 2, :],
                                              in_=vps[:, :, 1, :])

                # V32G[32r+m][g, o] = VN[32r+g][m, o]: DVE 32x32 block
                # transpose with a strided-read AP gives [m][o, g] chunks;
                # a GpSimd gather pass reorders each chunk to g-major so the
                # phase-C rhs is contiguous. Chunked per 64 o and emission-
                # interleaved with phases B/C so the DVE FIFO never blocks
                # the softmax chain.
                V32Gt = [v32p.tile([128, NK, O], BF16, tag="v32g",
                                   name=f"V32G_{h}_{sh}") for sh in range(2)]

                def v32t_chunk(sh, oc):
                    tmp = v32tmp.tile([128, 16, 32], BF16, tag="v32tmp")
                    nc.vector.transpose(
                        out=tmp,
                        in_=VN[sh][:, :, 16 * oc:16 * (oc + 1)].rearrange(
                            "p m o -> p o m"))
                    nc.gpsimd.tensor_copy(
                        out=V32Gt[sh][:, :, 16 * oc:16 * (oc + 1)],
                        in_=tmp.rearrange("p o g -> p g o"))

                for sh in range(2):
                    V32G = V32Gt[sh]
                    # ---- Phase B: logits + exp + rowsum + normalize ----
                    E = ep.tile([128, 32, NK], BF16, tag="E")   # [g, m]
                    rs = rsp.tile([128, 32], F32, tag="rs")
                    for gb in range(2):
                        lg = lg_ps.tile([128, 16, NK], F32, tag="lg")
                        for qi in range(16):
                            bl = sh * B_S + 16 * gb + qi
                            for t in range(2):
                                for j in range(4):
                                    nc.tensor.matmul(
                                        lg[32 * j:32 * (j + 1), qi, :],
                                        lhsT=QTs[:, :, t, bl + 32 * j],
                                        rhs=KTs[:, :, t, bl + 32 * j],
                                        start=(t == 0), stop=(t == 1),
                                        tile_position=(0, 32 * j),
                                        skip_group_check=True)
                        # softmax over m without max-subtraction: logits
                        # carry the 1/16 so |logit| <= ~2 and exp cannot
                        # overflow
                        sl = slice(16 * gb, 16 * gb + 16)
                        nc.scalar.activation(
                            out=E[:, sl, :].rearrange("p a b -> p (a b)"),
                            in_=lg.rearrange("p a b -> p (a b)"),
                            func=mybir.ActivationFunctionType.Exp)
                        nc.vector.reduce_sum(out=rs[:, sl], in_=E[:, sl, :],
                                             axis=mybir.AxisListType.X)
                        nc.vector.reciprocal(out=rs[:, sl], in_=rs[:, sl])
                        nc.vector.tensor_mul(
                            out=E[:, sl, :], in0=E[:, sl, :],
                            in1=rs[:, sl].unsqueeze(2).to_broadcast(
                                [128, 16, NK]))
                        if sh == 0:
                            # own V32G chunks between the softmax chains
                            for oc in range(8 * gb, 8 * gb + 8):
                                v32t_chunk(0, oc)

                    # ---- Phase C: attn @ value ----
                    for gc in range(4):
                        te8 = tep.tile([128, 8, NK], BF16, tag="te8")
                        nc.vector.transpose(
                            out=te8.rearrange("p a b -> p (a b)"),
                            in_=E[:, 8 * gc:8 * gc + 8, :].rearrange(
                                "p a b -> p (a b)"))
                        if sh == 0:
                            # sub-half 1's V32G chunks ride C(sh0)'s idle DVE
                            for oc in range(4 * gc, 4 * gc + 4):
                                v32t_chunk(1, oc)
                        for gq in range(2):
                            av = vp_ps.tile([128, 4, O], F32, tag="vp",
                                            name="av")
                            for gg in range(4):
                                gi = 4 * gq + gg
                                for j in range(4):
                                    nc.tensor.matmul(
                                        av[32 * j:32 * (j + 1), gg, :],
                                        lhsT=te8[32 * j:32 * (j + 1), gi, :],
                                        rhs=V32G[32 * j:32 * (j + 1),
                                                 8 * gc + gi, :],
                                        start=True, stop=True,
                                        tile_position=(32 * j, 32 * j),
                                        skip_group_check=True)
                            for gp in range(2):
                                OUTo = outp.tile([128, 2, O], BF16,
                                                 tag="OUTo")
                                nc.scalar.copy(
                                    out=OUTo, in_=av[:, 2 * gp:2 * gp + 2, :])
                                g4 = 4 * gq + 2 * gp
                                nc.gpsimd.dma_start(
                                    out=outD[h, sh, gc][:, g4:g4 + 2, :],
                                    in_=OUTo)
    return nc


def _prep_inputs(q, k, query_weight, key_weight, value_weight):
    bf = ml_dtypes.bfloat16
    # combo[h, slot, p, kind, c, x]; kind 0=qT, 1=kT, 2=qw/16
    def pack_qk(x, core):
        # x[32, 4096, 256] -> core slice -> [h, slot, p, c, b]
        xc = x[:, core * BS_CORE:(core + 1) * BS_CORE, :]
        r = xc.reshape(NQ, 2, B_H, 2, 128)        # [s, h, b, c, p]
        return r.transpose(1, 0, 4, 3, 2)          # [h, s, p, c, b]

    qwph = (np.asarray(query_weight) / 16.0).reshape(NQ, 2, 128, A)
    qwph = np.ascontiguousarray(qwph.transpose(0, 2, 1, 3)).astype(bf)

    kv = np.stack([np.asarray(key_weight), np.asarray(value_weight)], axis=2)
    # kv[s, d, w, a] -> [g4, p, s4, c, w, a]
    kvp = kv.reshape(8, 4, 2, 128, 2, A).transpose(0, 3, 1, 2, 4, 5)
    kvp = np.ascontiguousarray(kvp).astype(bf)

    in_maps = []
    for i in range(N_CORES):
        qp = pack_qk(np.asarray(q), i)
        kp = pack_qk(np.asarray(k), i)
        combo = np.empty((2, NQ, 128, 2, 2, 256), dtype=bf)
        combo[:, :, :, 0] = qp
        combo[:, :, :, 1] = kp
        in_maps.append({"combo": combo, "qw": qwph, "kvw": kvp})
    return in_maps


def _unpack_out(outs):
    # per-core out [2 h, 2 sh, 4 gc, 128(32j+n), 8 gg, 256] ->
    # [nq, 512, 256] with b = h*256 + sh*128 + 32j + 8*gc + gg
    full = []
    for o in outs:
        od = np.asarray(o).reshape(2, 2, 4, 4, 32, 8, O)
        full.append(od.transpose(4, 0, 1, 3, 2, 5, 6).reshape(NQ, BS_CORE, O))
    return np.concatenate(full, axis=1).astype(np.float32)


_NC_CACHE = {}


def _get_nc():
    if "nc" not in _NC_CACHE:
        nc = build_kernel()
        nc.finalize()
        _NC_CACHE["nc"] = nc
    return _NC_CACHE["nc"]


def kernel(q, k, query_weight, key_weight, value_weight, _trace=False):
    nc = _get_nc()
    in_maps = _prep_inputs(q, k, query_weight, key_weight, value_weight)
    res = run_bass_kernel_spmd(nc, in_maps, core_ids=list(range(N_CORES)),
                               trace=_trace)
    full = _unpack_out([res.results[i]["out"] for i in range(N_CORES)])
    if _trace:
        return full, res
    return full
